# revision 1
# baseline (speedup 1.0000x reference)
"""YOLOv1 loss kernel for Trainium2, data-parallel over 8 NeuronCores.

Full inputs: pred [16384,30,7,7] f32, labels [16384,30,7,7] f32 -> scalar f32.

Sharding: batch 16384 -> 8 cores x 2048 rows. Per core the kernel streams
pred [2048,1470] and a host-packed labels tensor [2048,1225] (channels 0-4
and 10-29; channels 5-9 are exact duplicates / unused in the reference),
computes the per-cell loss fully on-chip and reduces to [128, NCHUNK]
partial sums. Host sums the 8*128*NCHUNK partials and divides by B.

Math notes (all equivalent to the reference up to f32 rounding):
  - The grid offsets m,n cancel inside the IOU (equal shift of both boxes),
    and scaling all coords by 7 cancels in inter/union, so
    lo = x - 3.5w, hi = x + 3.5w, inter_raw = 49*inter,
    den = 49*(a1+ag) - inter_raw, iou = inter_raw/den.
  - a = w*h equals the reference's (x2-x1)*(y2-y1).
  - den >= 49*ag - ulp > 0 always (labels w,h >= 0.05), so the where() guard
    in the reference is unnecessary: inter==0 already gives iou = 0/den = 0.
  - obj = labels[:,4] exactly (conf is exactly 0/1).
  - inner = U2 + resp*(U1-U2) + cls with U1 = 5c1 + o1 + 0.5o2,
    U2 = 5c2 + o2 + 0.5o1; cell = obj*(inner - sph) + sph,
    sph = 0.5*(p4^2+p9^2).
"""

import numpy as np

import concourse.bass as bass
import concourse.mybir as mybir
import concourse.tile as tile
from concourse import bacc
from concourse.bass_utils import run_bass_kernel_spmd

F32 = mybir.dt.float32
OP = mybir.AluOpType
AF = mybir.ActivationFunctionType

NCORES = 8
B = 16384
BLOC = B // NCORES        # 2048 rows per core
P = 128                   # SBUF partitions
K = 4                     # 128-row blocks processed per chunk
NBLK = BLOC // P          # 16
NCHUNK = NBLK // K        # 4
PREDW = 30 * 49           # 1470, host-permuted channel order (see PERM)
LABW = 29 * 49            # 1421: [lx lx ly ly lw lw lh lh obj cls*20]
W = K * 49                # 196: one channel across the K blocks

# host-side pred channel permutation: pairs the two boxes so every
# per-box op is one contiguous 3D access pattern:
# [x1 x2 y1 y2 w1 w2 h1 h2 c1 c2 cls...]
PERM = [0, 5, 1, 6, 2, 7, 3, 8, 4, 9] + list(range(10, 30))

SQ5 = float(np.float32(np.sqrt(5.0)))
ISQ2 = float(np.float32(np.sqrt(0.5)))


def _body(tc, pred_ap, labs_ap, out_ap):
    nc = tc.nc
    nv = nc.vector
    na = nc.scalar
    ng = nc.gpsimd

    # DRAM views: row index = chunk*K*P + blk*P + p ; DMA iterates [p, k, f].
    pred_r = pred_ap.rearrange("(c k p) f -> c p k f", c=NCHUNK, k=K, p=P)
    labs_r = labs_ap.rearrange("(c k p) f -> c p k f", c=NCHUNK, k=K, p=P)

    import contextlib
    ctx = contextlib.ExitStack()
    with ctx:
        inp = ctx.enter_context(tc.tile_pool(name="inp", bufs=2))
        med = ctx.enter_context(tc.tile_pool(name="med", bufs=1))
        sml = ctx.enter_context(tc.tile_pool(name="sml", bufs=2))
        opool = ctx.enter_context(tc.tile_pool(name="opool", bufs=1))

        acc = opool.tile([P, NCHUNK], F32)

        for c in range(NCHUNK):
            PT = inp.tile([P, K * PREDW], F32, tag="PT")
            LT = inp.tile([P, K * LABW], F32, tag="LT")
            nc.sync.dma_start(
                PT[:].rearrange("p (k f) -> p k f", k=K), pred_r[c])
            nc.sync.dma_start(
                LT[:].rearrange("p (k f) -> p k f", k=K), labs_r[c])

            # paired pred layout (PERM): [x1 x2 y1 y2 w1 w2 h1 h2 c1 c2 cls]
            PT3 = PT[:].rearrange("p (k f) -> p k f", k=K)
            LT3 = LT[:].rearrange("p (k f) -> p k f", k=K)

            p_xy = PT3[:, :, 0:196]       # x1 x2 y1 y2
            p_wh = PT3[:, :, 196:392]     # w1 w2 h1 h2
            p_w = PT3[:, :, 196:294]
            p_h = PT3[:, :, 294:392]
            p_cf = PT3[:, :, 392:490]     # c1 c2
            p_cls = PT3[:, :, 490:1470]
            # labels: [lx lx ly ly | lw lw lh lh | obj | cls]
            l_xy = LT3[:, :, 0:196]
            l_wh = LT3[:, :, 196:392]
            l_w = LT3[:, :, 196:294]
            l_h = LT3[:, :, 294:392]
            l_obj = LT3[:, :, 392:441]
            l_cls = LT3[:, :, 441:1421]

            def t2(name, cols, pool=med, dt=F32):
                # tile with 3D view [p, K, cols]
                t = pool.tile([P, K * cols], dt, tag=name)
                return t, t[:].rearrange("p (k f) -> p k f", k=K)

            # ---- boxes: lo = xy - 3.5*wh, hi = xy + 3.5*wh (coords x7) ----
            _, lo_p = t2("lo_p", 196)
            _, hi_p = t2("hi_p", 196)
            _, lo_g = t2("lo_g", 196)
            _, hi_g = t2("hi_g", 196)
            nv.scalar_tensor_tensor(lo_p, p_wh, -3.5, p_xy, OP.mult, OP.add)
            nv.scalar_tensor_tensor(hi_p, p_wh, 3.5, p_xy, OP.mult, OP.add)
            nv.scalar_tensor_tensor(lo_g, l_wh, -3.5, l_xy, OP.mult, OP.add)
            nv.scalar_tensor_tensor(hi_g, l_wh, 3.5, l_xy, OP.mult, OP.add)

            # ---- areas (unscaled, both gt copies): a = w*h ----
            _, aa = t2("aa", 98)     # a1 a2
            _, ag = t2("ag", 98)     # ag ag
            nv.tensor_tensor(aa, p_w, p_h, OP.mult)
            nv.tensor_tensor(ag, l_w, l_h, OP.mult)
            _, ss = t2("ss", 98)     # a_k + ag
            nv.tensor_tensor(ss, aa, ag, OP.add)

            # ---- intersection ----
            _, mx = t2("mx", 196)
            _, mn = t2("mn", 196)
            nv.tensor_tensor(mx, lo_p, lo_g, OP.max)
            nv.tensor_tensor(mn, hi_p, hi_g, OP.min)
            _, dd = t2("dd", 196)
            nv.tensor_tensor(dd, mn, mx, OP.subtract)
            na.activation(dd, dd, AF.Relu)
            _, ii = t2("ii", 98)     # inter_raw (x49): i1 i2
            nv.tensor_tensor(ii, dd[:, :, 0:98], dd[:, :, 98:196], OP.mult)

            # ---- iou = inter_raw / (49*(a+ag) - inter_raw) ----
            _, dn = t2("dn", 98)
            nv.scalar_tensor_tensor(dn, ss, 49.0, ii, OP.mult, OP.subtract)
            _, rc = t2("rc", 98)
            nv.reciprocal(rc, dn)
            _, io = t2("io", 98)
            nv.tensor_tensor(io, ii, rc, OP.mult)

            _, resp = t2("resp", 49, sml, dt=mybir.dt.int32)
            nv.tensor_tensor(resp, io[:, :, 0:49], io[:, :, 49:98], OP.is_ge)

            # ---- conf terms: objc_k = (p_conf_k - iou_k)^2 ----
            _, dcp = t2("dcp", 98)
            nv.tensor_tensor(dcp, p_cf, io, OP.subtract)
            na.activation(dcp, dcp, AF.Square)   # -> objc1 objc2

            # ---- coor terms (x5 folded into squares) ----
            _, dxy = t2("dxy", 196)
            nv.tensor_tensor(dxy, p_xy, l_xy, OP.subtract)
            na.activation(dxy, dxy, AF.Square, scale=SQ5)  # 5*(dxy)^2
            _, sp = t2("sp", 196)
            na.activation(sp, p_wh, AF.Sqrt)
            _, sl = t2("sl", 196)
            na.activation(sl, l_wh, AF.Sqrt)
            _, ee = t2("ee", 196)
            nv.tensor_tensor(ee, sp, sl, OP.subtract)
            na.activation(ee, ee, AF.Square, scale=SQ5)    # 5*(e)^2
            nv.tensor_tensor(dxy, dxy, ee, OP.add)         # g (in-place)
            _, cc = t2("cc", 98)
            nv.tensor_tensor(cc, dxy[:, :, 0:98], dxy[:, :, 98:196],
                             OP.add)                        # 5*coor1, 5*coor2

            # ---- cls = sum_c (p_c - l_c)^2 over 20 channels ----
            _, dk = t2("dk", 980)
            ng.tensor_tensor(dk, p_cls, l_cls, OP.subtract)
            na.activation(dk, dk, AF.Square)
            _, u1 = t2("u1", 490)
            ng.tensor_tensor(u1, dk[:, :, 0:490], dk[:, :, 490:980], OP.add)
            _, u2 = t2("u2", 196)
            ng.tensor_tensor(u2, u1[:, :, 0:196], u1[:, :, 196:392], OP.add)
            _, u3 = t2("u3", 98, sml)
            nv.tensor_tensor(u3, u2[:, :, 0:98], u2[:, :, 98:196], OP.add)
            _, u4 = t2("u4", 49, sml)
            nv.tensor_tensor(u4, u3[:, :, 0:49], u3[:, :, 49:98], OP.add)
            _, u5 = t2("u5", 49, sml)
            nv.tensor_tensor(u5, u1[:, :, 392:441], u1[:, :, 441:490], OP.add)
            _, cls = t2("cls", 49, sml)
            nv.tensor_tensor(cls, u4, u5, OP.add)

            # ---- combine: inner = sel(resp, U1, U2) + cls ----
            objc1 = dcp[:, :, 0:49]
            objc2 = dcp[:, :, 49:98]
            _, u1a = t2("u1a", 49, sml)
            nv.scalar_tensor_tensor(u1a, objc2, 0.5, objc1, OP.mult, OP.add)
            _, U1 = t2("U1", 49, sml)
            nv.tensor_tensor(U1, u1a, cc[:, :, 0:49], OP.add)
            _, u2a = t2("u2a", 49, sml)
            nv.scalar_tensor_tensor(u2a, objc1, 0.5, objc2, OP.mult, OP.add)
            _, U2 = t2("U2", 49, sml)
            nv.tensor_tensor(U2, u2a, cc[:, :, 49:98], OP.add)
            selU_t, selU = t2("selU", 49, sml)
            na.activation(selU, U2, AF.Copy)
            nv.copy_predicated(selU_t[:], resp, U1)
            _, inner = t2("inner", 49, sml)
            nv.tensor_tensor(inner, selU, cls, OP.add)

            # ---- cell = obj ? inner : 0.5*(c1^2+c2^2), then reduce ----
            _, hp = t2("hp", 98)
            na.activation(hp, p_cf, AF.Square, scale=ISQ2)  # 0.5*conf^2
            obj_t, obj_v = t2("obj", 49, sml, dt=mybir.dt.int32)
            na.activation(obj_v, l_obj, AF.Copy)
            cell_t, cell = t2("cell", 49, sml)
            nv.tensor_tensor(cell, hp[:, :, 0:49], hp[:, :, 49:98], OP.add)
            nv.copy_predicated(cell_t[:], obj_t[:], inner)
            nv.tensor_reduce(acc[:, c:c + 1], cell_t[:],
                             mybir.AxisListType.X, OP.add)

        nc.sync.dma_start(out_ap, acc[:])


_NC_CACHE = None


def build_nc():
    global _NC_CACHE
    if _NC_CACHE is not None:
        return _NC_CACHE
    nc = bacc.Bacc(
        "TRN2",
        target_bir_lowering=False,
        debug=False,
        enable_asserts=False,
        num_devices=NCORES,
    )
    pred = nc.dram_tensor("pred", [BLOC, PREDW], F32, kind="ExternalInput")
    labs = nc.dram_tensor("labs", [BLOC, LABW], F32, kind="ExternalInput")
    out = nc.dram_tensor("out", [P, NCHUNK], F32, kind="ExternalOutput")
    with tile.TileContext(nc) as tc:
        _body(tc, pred.ap(), labs.ap(), out.ap())
    nc.compile()
    _NC_CACHE = nc
    return nc


def make_in_maps(pred, labels):
    pred = np.asarray(pred, dtype=np.float32)
    labels = np.asarray(labels, dtype=np.float32)
    pred2 = np.ascontiguousarray(pred[:, PERM]).reshape(B, PREDW)
    # labels: [lx lx ly ly | lw lw lh lh | obj | cls] (gt dup'd per box)
    lab2 = np.ascontiguousarray(
        labels[:, [0, 0, 1, 1, 2, 2, 3, 3, 4] + list(range(10, 30))]
    ).reshape(B, LABW)
    return [
        {
            "pred": np.ascontiguousarray(pred2[i * BLOC:(i + 1) * BLOC]),
            "labs": np.ascontiguousarray(lab2[i * BLOC:(i + 1) * BLOC]),
        }
        for i in range(NCORES)
    ]


def run(pred, labels, trace=False, **kw):
    nc = build_nc()
    in_maps = make_in_maps(pred, labels)
    res = run_bass_kernel_spmd(
        nc, in_maps, core_ids=list(range(NCORES)), trace=trace, **kw)
    total = np.float64(0.0)
    for r in res.results:
        total += r["out"].astype(np.float64).sum()
    loss = np.float32(total / B)
    return loss, res


def kernel(pred, labels):
    loss, _ = run(pred, labels)
    return np.array(loss, dtype=np.float32)



# revision 15
# speedup vs baseline: 3.5778x; 3.5778x over previous
"""YOLOv1 loss kernel for Trainium2, data-parallel over 8 NeuronCores.

Full inputs: pred [16384,30,7,7] f32, labels [16384,30,7,7] f32 -> scalar f32.

Strategy (v1 "compact"):
  Each grid cell (row, i, j) is an independent unit: the loss is a plain sum
  of per-cell terms, and the grid offsets m,n cancel inside the IOU (both
  boxes of a cell shift equally). Cells split into two streams:
    - obj cells (labels[:,4]==1, ~30%): full pipeline (IOU, responsibility,
      coor, conf, cls) over 58 bf16 values/cell, box-major:
      [x1 y1 w1 h1 c1 | x2 y2 w2 h2 c2 | pcls*20 | (lx ly lw lh)*2 | lcls*20]
      (label box channels duplicated so every op is a 2D/3D access pattern).
    - noobj cells (~70%): only 0.5*(c1^2 + c2^2), i.e. 2 bf16 values/cell.
  The host packs each stream densely over 128 partitions (bf16 casts,
  channel gather/duplication, padding with exactly-zero-contribution cells);
  all loss arithmetic runs on device. Per-partition f32 partials are summed
  on host (f64) and divided by B.

Math notes (equivalent to the reference up to bf16 rounding):
  - 1D overlap identity: min(a+p, b+q) - max(a-p, b-q) = (p+q) - max(|a-b|,
    |p-q|); with p=3.5*pw, q=3.5*gw this gives the (x7-scaled) intersection
    side from dxy = px-gx (shared with the coor term) and 3.5-scaled widths.
  - ii = 49*inter, dn = 49*(aa+ag) - ii > 0 always (label w,h >= 0.05), so
    iou = ii/dn needs no zero-guard; 1/dn via reciprocal_approx_fast (f32).
  - On obj cells the conf coefficient of (c_k - iou_k)^2 is resp_k + 0.5*
    (1-resp_k) = 0.5 + 0.5*resp_k; coor coefficient is 5*resp_k; cls
    unweighted. resp1 = iou1>=iou2, resp2 = 1-resp1.
  - tensor_tensor_reduce(out=(w*d)*d) accumulates sum(w*d^2) per partition
    in one DVE op (w folded into a prior multiply).
"""

import numpy as np
import ml_dtypes

import concourse.bass as bass
import concourse.mybir as mybir
import concourse.tile as tile
from concourse import bacc
from concourse.bass_utils import run_bass_kernel_spmd

F32 = mybir.dt.float32
BF16 = mybir.dt.bfloat16
OP = mybir.AluOpType
AF = mybir.ActivationFunctionType

NCORES = 8
B = 16384
BLOC = B // NCORES        # 2048 rows per core
S2 = 49
P = 128
NCH = 58                  # obj-stream channels per cell
NSL = 2                   # obj-stream slices (DMA/compute pipelining)

LAB_PERM = [0, 1, 2, 3] * 2 + list(range(10, 30))
# pad cell: identical unit boxes, conf=1 (=iou), zero cls -> contributes 0
PAD_CELL = np.array(
    [0, 0, 1, 1, 1] * 2 + [0] * 20 + [0, 0, 1, 1] * 2 + [0] * 20,
    dtype=np.float32,
)

BF = ml_dtypes.bfloat16


def _body(tc, xo_ap, xn_ap, out_ap, CW, CN):
    nc = tc.nc
    nv = nc.vector
    na = nc.scalar

    import contextlib
    ctx = contextlib.ExitStack()
    with ctx:
        inp = ctx.enter_context(tc.tile_pool(name="inp", bufs=2))
        med = ctx.enter_context(tc.tile_pool(name="med", bufs=2))
        opool = ctx.enter_context(tc.tile_pool(name="opool", bufs=1))

        acc = opool.tile([P, 4 * NSL + 1], F32)

        for s in range(NSL):
            X = inp.tile([P, NCH * CW], BF16, tag="X")
            nc.sync.dma_start(X[:], xo_ap[:, s * NCH * CW:(s + 1) * NCH * CW])

            # V: [p, 58 channels, CW]; pred boxes at channel offsets 0 and 5.
            V = X[:].rearrange("p (c w) -> p c w", c=NCH)
            pb = V[:, 0:10].rearrange("p (b c) w -> p b (c w)", b=2)
            pxy2 = pb[:, :, 0:2 * CW]           # [p, 2, (x y)*CW]
            pwh2 = pb[:, :, 2 * CW:4 * CW]      # [p, 2, (w h)*CW]
            pcf2 = pb[:, :, 4 * CW:5 * CW]      # [p, 2, c*CW]
            p_cls = V[:, 10:30].rearrange("p c w -> p (c w)")
            lb = V[:, 30:38].rearrange("p (b c) w -> p b (c w)", b=2)
            lxy2 = lb[:, :, 0:2 * CW]
            lwh2 = lb[:, :, 2 * CW:4 * CW]
            l_cls = V[:, 38:58].rearrange("p c w -> p (c w)")

            def t(name, n, dt=BF16):
                # flat [P, n*CW]
                tt = med.tile([P, n * CW], dt, tag=name, name=name)
                return tt[:]

            scr = med.tile([P, 20 * CW], BF16, tag="scr", name="scr")

            # ---- coor xy diff (also feeds the IOU overlap identity) ----
            dxy = t("dxy", 4)     # [dx1 dy1 dx2 dy2]
            nv.tensor_tensor(dxy.rearrange("p (b r) -> p b r", b=2),
                             pxy2, lxy2, OP.subtract)

            # ---- IOU: inter = (p+q) - max(|dxy|,|p-q|), scaled by 7 ----
            l35 = t("l35", 4)     # [lw lh lw lh] * 3.5
            nv.tensor_scalar_mul(
                l35.rearrange("p (b r) -> p b r", b=2),
                lwh2, 3.5)
            s35 = t("s35", 4)
            nv.scalar_tensor_tensor(
                s35.rearrange("p (b r) -> p b r", b=2), pwh2, 3.5,
                l35.rearrange("p (b r) -> p b r", b=2), OP.mult, OP.add)
            e35 = t("e35", 4)
            nv.scalar_tensor_tensor(
                e35.rearrange("p (b r) -> p b r", b=2), pwh2, 3.5,
                l35.rearrange("p (b r) -> p b r", b=2), OP.mult, OP.subtract)
            adel = t("adel", 4)
            na.activation(adel, dxy, AF.Abs)
            aee = t("aee", 4)
            na.activation(aee, e35, AF.Abs)
            m2 = t("m2", 4)
            nv.tensor_tensor(m2, adel, aee, OP.max)
            side = t("side", 4)   # [sx1 sy1 sx2 sy2]
            nv.tensor_tensor(side, s35, m2, OP.subtract)
            nv.tensor_scalar_max(side, side, 0.0)
            sv = side.rearrange("p (b cr) -> p b cr", b=2)  # [p,2,(x y)CW]
            ii = t("ii", 2)       # [i1 i2]
            nv.tensor_tensor(ii.rearrange("p (b r) -> p b r", b=2),
                             sv[:, :, 0:CW], sv[:, :, CW:2 * CW], OP.mult)

            aa = t("aa", 2)       # [a1 a2]
            nv.tensor_tensor(aa.rearrange("p (b r) -> p b r", b=2),
                             pwh2[:, :, 0:CW], pwh2[:, :, CW:2 * CW],
                             OP.mult)
            ag = t("ag", 1)
            nv.tensor_tensor(ag, l35[:, 0:CW], l35[:, CW:2 * CW], OP.mult)
            A49 = t("A49", 2)
            nv.scalar_tensor_tensor(A49, aa, 49.0, ii, OP.mult, OP.subtract)
            dn = t("dn", 2, dt=F32)
            nv.scalar_tensor_tensor(
                dn.rearrange("p (b r) -> p b r", b=2),
                ag.unsqueeze(1).broadcast_to((P, 2, CW)),
                4.0, A49.rearrange("p (b r) -> p b r", b=2),
                OP.mult, OP.add)
            rc = t("rc", 2, dt=F32)
            nv.reciprocal_approx_fast(rc, dn)
            io = t("io", 2)       # [io1 io2]
            nv.tensor_tensor(io, ii, rc, OP.mult)

            # ---- responsibility + weights ----
            wt = t("wt", 2)       # [resp1 resp2]
            nv.tensor_tensor(wt[:, 0:CW], io[:, 0:CW], io[:, CW:2 * CW],
                             OP.is_ge)
            nv.tensor_scalar(wt[:, CW:2 * CW], wt[:, 0:CW], -1.0, 1.0,
                             OP.mult, OP.add)
            g2 = t("g2", 2)       # [5*resp1 5*resp2]
            nv.tensor_scalar_mul(g2, wt, 5.0)
            w2 = t("w2", 2)       # [.5+.5*resp1 .5+.5*resp2]
            nv.tensor_scalar(w2, wt, 0.5, 0.5, OP.mult, OP.add)

            # ---- conf: sum w2 * (c - iou)^2 ----
            t2 = t("t2", 2)
            nv.tensor_tensor(t2.rearrange("p (b r) -> p b r", b=2),
                             pcf2, io.rearrange("p (b r) -> p b r", b=2),
                             OP.subtract)
            cw_ = t("cw", 2)
            nv.tensor_tensor(cw_, t2, w2, OP.mult)
            nv.scalar_tensor_tensor(
                scr[:, 0:2 * CW], cw_, 1.0, t2, OP.mult, OP.mult,
                accum_out=acc[:, 4 * s + 0:4 * s + 1])

            # ---- coor xy: sum 5*resp * dxy^2 (per box, g2 bcast) ----
            mxy = t("mxy", 4)
            for b_ in range(2):
                nv.tensor_tensor(
                    mxy[:, 2 * b_ * CW:2 * (b_ + 1) * CW].rearrange(
                        "p (c r) -> p c r", c=2),
                    dxy[:, 2 * b_ * CW:2 * (b_ + 1) * CW].rearrange(
                        "p (c r) -> p c r", c=2),
                    g2[:, b_ * CW:(b_ + 1) * CW].unsqueeze(1)
                    .broadcast_to((P, 2, CW)),
                    OP.mult)
            nv.scalar_tensor_tensor(
                scr[:, 0:4 * CW], mxy, 1.0, dxy, OP.mult, OP.mult,
                accum_out=acc[:, 4 * s + 1:4 * s + 2])

            # ---- coor wh: sum 5*resp * (sqrt(pwh)-sqrt(lwh))^2 ----
            sp = t("sp", 4)       # [sw1 sh1 sw2 sh2]
            na.activation(sp.rearrange("p (b r) -> p b r", b=2), pwh2,
                          AF.Sqrt)
            sl = t("sl", 4)       # [slw slh slw slh]
            na.activation(sl.rearrange("p (b r) -> p b r", b=2), lwh2,
                          AF.Sqrt)
            ds = t("ds", 4)
            nv.tensor_tensor(ds, sp, sl, OP.subtract)
            mwh = t("mwh", 4)
            for b_ in range(2):
                nv.tensor_tensor(
                    mwh[:, 2 * b_ * CW:2 * (b_ + 1) * CW].rearrange(
                        "p (c r) -> p c r", c=2),
                    ds[:, 2 * b_ * CW:2 * (b_ + 1) * CW].rearrange(
                        "p (c r) -> p c r", c=2),
                    g2[:, b_ * CW:(b_ + 1) * CW].unsqueeze(1)
                    .broadcast_to((P, 2, CW)),
                    OP.mult)
            nv.scalar_tensor_tensor(
                scr[:, 0:4 * CW], mwh, 1.0, ds, OP.mult, OP.mult,
                accum_out=acc[:, 4 * s + 2:4 * s + 3])

            # ---- cls: sum (pcls - lcls)^2 ----
            dc = t("dc", 20)
            nv.tensor_tensor(dc, p_cls, l_cls, OP.subtract)
            nv.scalar_tensor_tensor(
                scr[:], dc, 1.0, dc, OP.mult, OP.mult,
                accum_out=acc[:, 4 * s + 3:4 * s + 4])

        # ---- noobj stream: sum 0.5*(c1^2 + c2^2) ----
        XN = inp.tile([P, 2 * CN], BF16, tag="XN")
        nc.sync.dma_start(XN[:], xn_ap)
        scrn = inp.tile([P, 2 * CN], BF16, tag="scrn", name="scrn")
        nv.scalar_tensor_tensor(
            scrn[:], XN[:], 0.5, XN[:], OP.mult, OP.mult,
            accum_out=acc[:, 4 * NSL:4 * NSL + 1])

        nc.sync.dma_start(out_ap, acc[:])


_NC_CACHE = {}


def build_nc(CW, CN):
    key = (CW, CN)
    if key in _NC_CACHE:
        return _NC_CACHE[key]
    nc = bacc.Bacc(
        "TRN2",
        target_bir_lowering=False,
        debug=False,
        enable_asserts=False,
        num_devices=NCORES,
    )
    xo = nc.dram_tensor("xo", [P, NSL * NCH * CW], BF16, kind="ExternalInput")
    xn = nc.dram_tensor("xn", [P, 2 * CN], BF16, kind="ExternalInput")
    out = nc.dram_tensor("out", [P, 4 * NSL + 1], F32, kind="ExternalOutput")
    with tile.TileContext(nc) as tc:
        _body(tc, xo.ap(), xn.ap(), out.ap(), CW, CN)
    nc.compile()
    _NC_CACHE[key] = nc
    return nc


def _pack_core(Pc, Lc, m, CW, CN):
    """Pc,Lc: [BLOC*49, 30] f32 per-cell channels; m: bool obj mask."""
    idx1 = np.nonzero(m)[0]
    idx0 = np.nonzero(~m)[0]
    nco = NSL * CW
    O = np.empty((nco * P, NCH), dtype=np.float32)
    k1 = len(idx1)
    O[:k1, 0:30] = Pc[idx1]
    O[:k1, 30:58] = Lc[idx1][:, LAB_PERM]
    O[k1:] = PAD_CELL
    # cell j -> (col q=j//P, p=j%P); col q -> (slice s=q//CW, w=q%CW)
    xo = O.reshape(NSL, CW, P, NCH).transpose(2, 0, 3, 1)
    xo = np.ascontiguousarray(xo).reshape(P, NSL * NCH * CW).astype(BF)

    k0 = len(idx0)
    N = np.zeros((CN * P, 2), dtype=np.float32)
    N[:k0, 0] = Pc[idx0, 4]
    N[:k0, 1] = Pc[idx0, 9]
    xn = N.reshape(CN, P, 2).transpose(1, 2, 0)
    xn = np.ascontiguousarray(xn).reshape(P, 2 * CN).astype(BF)
    return {"xo": xo, "xn": xn}


def prepare(pred, labels):
    pred = np.asarray(pred, dtype=np.float32).reshape(B, 30, S2)
    labels = np.asarray(labels, dtype=np.float32).reshape(B, 30, S2)
    masks = []
    Pcs = []
    Lcs = []
    k1s = []
    for c in range(NCORES):
        r0 = c * BLOC
        Pc = np.ascontiguousarray(
            pred[r0:r0 + BLOC].transpose(0, 2, 1)).reshape(-1, 30)
        Lc = np.ascontiguousarray(
            labels[r0:r0 + BLOC].transpose(0, 2, 1)).reshape(-1, 30)
        m = Lc[:, 4] == 1.0
        masks.append(m)
        Pcs.append(Pc)
        Lcs.append(Lc)
        k1s.append(int(m.sum()))
    k1max = max(k1s)
    k0max = max(BLOC * S2 - k1 for k1 in k1s)

    # obj cols per slice (even, so channel blocks stay 4B aligned)
    def cdiv(a, b):
        return -(-a // b)

    CW = max(2, cdiv(cdiv(cdiv(k1max, P), NSL), 2) * 2)
    CN = max(2, cdiv(cdiv(k0max, P), 2) * 2)
    nc = build_nc(CW, CN)
    in_maps = [
        _pack_core(Pcs[c], Lcs[c], masks[c], CW, CN) for c in range(NCORES)
    ]
    return nc, in_maps


def run(pred, labels, trace=False, **kw):
    nc, in_maps = prepare(pred, labels)
    res = run_bass_kernel_spmd(
        nc, in_maps, core_ids=list(range(NCORES)), trace=trace, **kw)
    total = np.float64(0.0)
    for r in res.results:
        total += r["out"].astype(np.float64).sum()
    loss = np.float32(total / B)
    return loss, res


def kernel(pred, labels):
    loss, _ = run(pred, labels)
    return np.array(loss, dtype=np.float32)
